# revision 52
# baseline (speedup 1.0000x reference)
"""Multi-head causal attention (B=2, T=2048, D=1024, H=16) on 8 trn2 cores.

Sharding: core c = (batch b, head-group g) with b = c//4, g = c%4.
Each core computes Q/K/V projections for its 4 heads (256 features),
causal attention, and its partial output projection; the host sums the
4 per-batch partials (the w_o all-reduce) and stacks batches.

v2: q-stripe-major attention with head-paired matmuls.
  - S matmuls for the head pair (partitions 0:64 / 64:128) are row-tiled
    (tile_position (0,0)/(64,0)) and run concurrently on the PE array.
  - PV and denominator matmuls are col-tiled pairs ((0,0)/(0,64)).
  - The softmax denominator comes from a ones-lhsT matmul broadcast to
    64 psum rows per head, so normalization is recip+mult (no shuffles).
  - Projections / attention / output projection are emission-interleaved
    so scalar(exp) work starts ~10us in rather than after all projections.
  - Output partials are written f16 (halves the out DMA).
"""

import math

import ml_dtypes
import numpy as np

BF16NP = ml_dtypes.bfloat16

import concourse.bass as bass
from concourse import bacc
import concourse.mybir as mybir
import concourse.tile as tile
from concourse.bass_utils import run_bass_kernel_spmd

F32 = mybir.dt.float32
F16 = mybir.dt.float16
AF = mybir.ActivationFunctionType
ALU = mybir.AluOpType
BF16 = mybir.dt.bfloat16

B, T, D, H = 2, 2048, 1024, 16
NCORES = 8
G = 4             # head groups (tensor parallel); cores = B * G
HPG = H // G      # 4 heads per core
DH = D // H       # 64 head dim
E = D // G        # 256 features per core
EB = E // 128     # 2 e-blocks of 128 (= head pairs)
KD = D // 128     # 8 contraction chunks for projections
TN = T // 512     # 4 512-wide stripes
TC = T // 128     # 16 128-wide k/t chunks


def build_nc():
    nc = bacc.Bacc(None)
    xqT = nc.declare_dram_parameter("xqT", [D, T], BF16, isOutput=False)
    xkT = nc.declare_dram_parameter("xkT", [D, T], BF16, isOutput=False)
    xvT = nc.declare_dram_parameter("xvT", [D, T], BF16, isOutput=False)
    wqT = nc.declare_dram_parameter("wqT", [D, E], BF16, isOutput=False)
    wkT = nc.declare_dram_parameter("wkT", [D, E], BF16, isOutput=False)
    wvT = nc.declare_dram_parameter("wvT", [D, E], BF16, isOutput=False)
    woT = nc.declare_dram_parameter("woT", [E, D], BF16, isOutput=False)
    outp = nc.declare_dram_parameter("outp", [T, D], F16, isOutput=True)

    with tile.TileContext(nc) as tc:
        with (
            tc.tile_pool(name="persist", bufs=1) as persist,
            tc.tile_pool(name="xs", bufs=12) as xs,
            tc.tile_pool(name="pt", bufs=12) as ptp,
            tc.tile_pool(name="rs", bufs=6) as rsp,
            tc.tile_pool(name="outs", bufs=4) as outsp,
            tc.tile_pool(name="psa", bufs=1, space="PSUM") as psa,
            tc.tile_pool(name="psb", bufs=1, space="PSUM") as psb,
            tc.tile_pool(name="po", bufs=1, space="PSUM") as pop,
            tc.tile_pool(name="pd", bufs=1, space="PSUM") as pdp,
            tc.tile_pool(name="pj", bufs=2, space="PSUM") as pjp,
        ):
            wq_sb = persist.tile([128, KD, E], BF16, tag="wq")
            wk_sb = persist.tile([128, KD, E], BF16, tag="wk")
            wv_sb = persist.tile([128, KD, E], BF16, tag="wv")
            wo_sb = persist.tile([128, EB, D], BF16, tag="wo")
            ones64 = persist.tile([128, 64], BF16, tag="ones")

            # fine-grained persist tiles so interleaved consumers only
            # depend on the exact producer stripe
            QTn = [[persist.tile([128, 512], BF16, tag=f"qt{eb}{n}", name=f"qt{eb}{n}")
                    for n in range(TN)] for eb in range(EB)]
            KTn = [[persist.tile([128, 512], BF16, tag=f"kt{eb}{n}", name=f"kt{eb}{n}")
                    for n in range(TN)] for eb in range(EB)]
            ONn = [[persist.tile([128, 512], BF16, tag=f"on{eb}{n}", name=f"on{eb}{n}")
                    for n in range(TN)] for eb in range(EB)]
            Vpn = [persist.tile([128, 4, HPG, DH], BF16, tag=f"vp{n}", name=f"vp{n}")
                   for n in range(TN)]

            nc.vector.memset(ones64[:], 1.0)
            # dummy exp: pulls the ACT table load off the first real
            # exp call's critical path
            dum = persist.tile([1, 2], BF16, tag="dum")
            nc.scalar.activation(dum[:, :], ones64[0:1, 0:2], AF.Exp)
            # HAM warm-up: keep the PE busy through the input-DMA window so
            # its clock gate is at 8/8 (2.4 GHz) when the first projection
            # matmuls arrive (the activity window needs ~3.4us of load)
            wup = pjp.tile([128, 64], F32, tag="pj")
            for _ in range(80):
                nc.tensor.matmul(wup[0:64, :], ones64[:, :], ones64[:, :],
                                 start=True, stop=True)

            def emit_x_dma(xdram, n):
                t = xs.tile([128, KD, 512], BF16, tag="x")
                nc.sync.dma_start(
                    t[:], xdram[:, :].rearrange("(c p) t -> p c t", p=128)[
                        :, :, 512 * n : 512 * n + 512]
                )
                return t

            def emit_qk_proj(eb, n, xt, wsb, dest):
                # medium priority (below S, above PV/den/out-proj backlog):
                # these produce the next attention block's inputs and must
                # not drain behind slack work
                acc = pjp.tile([128, 512], F32, tag="pj")
                with tc.high_priority(offset=25000):
                    for kd in range(KD):
                        nc.tensor.matmul(
                            acc[:],
                            wsb[:, kd, 128 * eb : 128 * eb + 128],
                            xt[:, kd, :],
                            start=(kd == 0),
                            stop=(kd == KD - 1),
                        )
                    nc.vector.tensor_copy(dest[eb][n][:, :], acc[:])

            def emit_v_proj_sub(n, sub, xt):
                acc = pjp.tile([128, E], F32, tag="pj")
                for kd in range(KD):
                    nc.tensor.matmul(
                        acc[:],
                        xt[:, kd, 128 * sub : 128 * sub + 128],
                        wv_sb[:, kd, :],
                        start=(kd == 0),
                        stop=(kd == KD - 1),
                    )
                nc.vector.tensor_copy(
                    Vpn[n][:, sub, :, :],
                    acc[:].rearrange("p (h d) -> p h d", h=HPG),
                )

            def emit_attn_jg(eb, jg, filler):
                """Attention for head pair eb, query stripe jg (512 q's),
                accumulating over k-chunks 0..4*jg+3 in pairs."""
                nkc = 4 * jg + 4
                pO = pop.tile([128, 512], F32, tag="po")
                pD = pdp.tile([128, 512], F32, tag="pd")

                # one k-chunk per step: pS is 2 banks so the pool's 2 bufs
                # let S(s+1) run during exp(s) — no exp<->S serial chain.
                # One exp call covers both heads, so all four col matmuls of
                # a step are co-ready: PV_h pairs with den of the OTHER head
                # (disjoint PE col groups) and the scheduler pops them
                # concurrently.  S is emitted one step ahead of the col
                # waves so the scalar engine's next input never queues
                # behind PV/den/filler work on the PE.
                def emit_S(kc):
                    # alternate between two single-buffer pools so each
                    # step's WAR dep is exactly exp(s-2), not batched with
                    # the other slot's chain
                    pool = psa if kc % 2 == 0 else psb
                    pS = pool.tile([128, 1024], F32, tag="ps", name="pS")
                    # highest scheduler priority: S feeds the scalar engine
                    # (exp), which must never starve behind projection /
                    # PV backlog on the PE; data deps still gate execution
                    with tc.high_priority(offset=50000):
                        for h in range(2):
                            r0 = 64 * h
                            nc.tensor.matmul(
                                pS[:, 512 * h : 512 * h + 512],
                                KTn[eb][kc // 4][r0 : r0 + 64,
                                                 128 * (kc % 4) : 128 * (kc % 4) + 128],
                                QTn[eb][jg][r0 : r0 + 64, :],
                                start=True,
                                stop=True,
                            )
                    return pS

                pS_next = emit_S(0)
                for kc in range(nkc):
                    pS = pS_next
                    ptb = ptp.tile([128, 1024], BF16, tag="pt")
                    nc.scalar.activation(ptb[:, :], pS[:, :], AF.Exp)
                    if kc >= 4 * jg:  # diagonal chunk: causal mask in place
                        # one call over both heads (outer level restarts the
                        # q ramp per 512-half) so they stay co-ready
                        nc.gpsimd.affine_select(
                            out=ptb[:, :].rearrange("p (h q) -> p h q", h=2),
                            in_=ptb[:, :].rearrange("p (h q) -> p h q", h=2),
                            pattern=[[0, 2], [1, 512]],
                            compare_op=ALU.is_ge,
                            fill=0.0,
                            base=-(128 * (kc % 4)),
                            channel_multiplier=-1,
                        )
                    if kc + 1 < nkc:
                        pS_next = emit_S(kc + 1)

                    def pv_mm(h):
                        nc.tensor.matmul(
                            pO[64 * h : 64 * h + 64, :],
                            Vpn[kc // 4][:, kc % 4, 2 * eb + h, :],
                            ptb[:, 512 * h : 512 * h + 512],
                            start=(kc == 0),
                            stop=(kc == nkc - 1),
                            skip_group_check=True,
                        )
                    def den_mm(h):
                        nc.tensor.matmul(
                            pD[64 * h : 64 * h + 64, :],
                            ones64[:, :],
                            ptb[:, 512 * h : 512 * h + 512],
                            start=(kc == 0),
                            stop=(kc == nkc - 1),
                            skip_group_check=True,
                        )
                    pv_mm(0); den_mm(1)
                    pv_mm(1); den_mm(0)
                    if filler:
                        filler.pop(0)()
                # normalization: recip of broadcast denominator, then scale.
                # fast 51-ULP recip (1 DVE inst) is plenty for the 2e-2
                # budget.  The last block's norm is split in halves so the
                # tail out-projections start as soon as their half lands.
                r = rsp.tile([128, 512], F32, tag="rs")
                halves = 2 if (eb, jg) == (1, 3) else 1
                w = 512 // halves
                for hv in range(halves):
                    sl = slice(w * hv, w * hv + w)
                    nc.vector.reciprocal_approx_fast(out=r[:, sl], in_=pD[:, sl])
                    nc.vector.tensor_tensor(
                        out=ONn[eb][jg][:, sl], in0=pO[:, sl], in1=r[:, sl],
                        op=ALU.mult,
                    )

            def emit_outproj(tn, tail=False):
                obt = outsp.tile([128, 1024], F16, tag="ob")
                for dn in range(2):
                    acc = pjp.tile([128, 512], F32, tag="pj")
                    for eb in range(EB):
                        nc.tensor.matmul(
                            acc[:],
                            ONn[eb][tn // 4][:, 128 * (tn % 4) : 128 * (tn % 4) + 128],
                            wo_sb[:, eb, 512 * dn : 512 * dn + 512],
                            start=(eb == 0),
                            stop=(eb == EB - 1),
                        )
                    if tail and dn == 1:
                        # after the last exp, the scalar engine is free:
                        # parallelize the final casts across engines
                        nc.scalar.copy(obt[:, 512:1024], acc[:])
                    else:
                        nc.vector.tensor_copy(obt[:, 512 * dn : 512 * dn + 512], acc[:])
                nc.sync.dma_start(
                    outp[128 * tn : 128 * tn + 128, :], obt[:]
                )

            # ---- emission schedule ----
            # DMA order front-loads exactly what the first projections need
            nc.sync.dma_start(wq_sb[:], wqT[:, :].rearrange("(c p) e -> p c e", p=128))
            xq0 = emit_x_dma(xqT, 0)
            nc.sync.dma_start(wk_sb[:], wkT[:, :].rearrange("(c p) e -> p c e", p=128))
            xk0 = emit_x_dma(xkT, 0)
            nc.sync.dma_start(wv_sb[:], wvT[:, :].rearrange("(c p) e -> p c e", p=128))
            xv0 = emit_x_dma(xvT, 0)
            nc.sync.dma_start(wo_sb[:], woT[:, :].rearrange("(c p) d -> p c d", p=128))
            emit_qk_proj(0, 0, xq0, wq_sb, QTn)
            emit_qk_proj(0, 0, xk0, wk_sb, KTn)
            for sub in range(4):
                emit_v_proj_sub(0, sub, xv0)
            # prefetch the remaining stripes now: the DMA queue streams them
            # during stripe-0 compute so later projections never wait
            xst = {}
            for n in range(1, TN):
                xst[n] = (emit_x_dma(xqT, n), emit_x_dma(xkT, n),
                          emit_x_dma(xvT, n))

            # interleave head pairs per q-stripe; ALL remaining projection
            # and out-projection work rides as fillers inside attention
            # steps so no emission block ever starves the scalar engine
            op = lambda t: (lambda: emit_outproj(t))
            def q0(m):
                return lambda: emit_qk_proj(0, m, xst[m][0], wq_sb, QTn)
            def k0(m):
                return lambda: emit_qk_proj(0, m, xst[m][1], wk_sb, KTn)
            def q1(m):
                x = xq0 if m == 0 else xst[m][0]
                return lambda: emit_qk_proj(1, m, x, wq_sb, QTn)
            def k1(m):
                x = xk0 if m == 0 else xst[m][1]
                return lambda: emit_qk_proj(1, m, x, wk_sb, KTn)
            def vp(m, s0):
                return lambda: (emit_v_proj_sub(m, s0, xst[m][2]),
                                emit_v_proj_sub(m, s0 + 1, xst[m][2]))

            emit_attn_jg(0, 0, [q1(0), k1(0)])
            emit_attn_jg(1, 0, [q0(1), k0(1), vp(1, 0), vp(1, 2)])
            emit_attn_jg(0, 1, [q1(1), k1(1)])
            emit_attn_jg(1, 1, [q0(2), k0(2), op(0), op(1), op(2), op(3),
                                vp(2, 0), vp(2, 2)])
            emit_attn_jg(0, 2, [q1(2), k1(2)])
            emit_attn_jg(1, 2, [q0(3), k0(3), op(4), op(5), op(6), op(7),
                                vp(3, 0), vp(3, 2)])
            emit_attn_jg(0, 3, [q1(3), k1(3)])
            emit_attn_jg(1, 3, [op(t) for t in range(8, 12)])
            for t in range(12, 16):
                emit_outproj(t, tail=True)
    nc.compile()
    return nc


_CACHE = {}
LAST_RESULTS = None


def get_nc():
    if "nc" not in _CACHE:
        _CACHE["nc"] = build_nc()
    return _CACHE["nc"]


def make_in_maps(q, k, v, wq, wk, wv, wo):
    q, k, v, wq, wk, wv, wo = (
        np.asarray(a, dtype=np.float32) for a in (q, k, v, wq, wk, wv, wo)
    )
    scale = 1.0 / math.sqrt(DH)
    xT = [
        (
            np.ascontiguousarray(q[b].T).astype(BF16NP),
            np.ascontiguousarray(k[b].T).astype(BF16NP),
            np.ascontiguousarray(v[b].T).astype(BF16NP),
        )
        for b in range(B)
    ]
    in_maps = []
    for c in range(NCORES):
        b, g = divmod(c, G)
        gs = slice(E * g, E * (g + 1))
        in_maps.append(
            {
                "xqT": xT[b][0],
                "xkT": xT[b][1],
                "xvT": xT[b][2],
                "wqT": np.ascontiguousarray((wq[gs] * scale).T).astype(BF16NP),
                "wkT": np.ascontiguousarray(wk[gs].T).astype(BF16NP),
                "wvT": np.ascontiguousarray(wv[gs].T).astype(BF16NP),
                "woT": np.ascontiguousarray(wo[:, gs].T).astype(BF16NP),
            }
        )
    return in_maps


def kernel(q, k, v, wq, wk, wv, wo):
    global LAST_RESULTS
    nc = get_nc()
    in_maps = make_in_maps(q, k, v, wq, wk, wv, wo)
    res = run_bass_kernel_spmd(nc, in_maps, core_ids=list(range(NCORES)))
    LAST_RESULTS = res
    out = np.zeros((B, T, D), dtype=np.float32)
    for c in range(NCORES):
        out[c // G] += np.asarray(res.results[c]["outp"], dtype=np.float32)
    return out


# revision 53
# speedup vs baseline: 1.0117x; 1.0117x over previous
"""Multi-head causal attention (B=2, T=2048, D=1024, H=16) on 8 trn2 cores.

Sharding: core c = (batch b, head-group g) with b = c//4, g = c%4.
Each core computes Q/K/V projections for its 4 heads (256 features),
causal attention, and its partial output projection; the host sums the
4 per-batch partials (the w_o all-reduce) and stacks batches.

v2: q-stripe-major attention with head-paired matmuls.
  - S matmuls for the head pair (partitions 0:64 / 64:128) are row-tiled
    (tile_position (0,0)/(64,0)) and run concurrently on the PE array.
  - PV and denominator matmuls are col-tiled pairs ((0,0)/(0,64)).
  - The softmax denominator comes from a ones-lhsT matmul broadcast to
    64 psum rows per head, so normalization is recip+mult (no shuffles).
  - Projections / attention / output projection are emission-interleaved
    so scalar(exp) work starts ~10us in rather than after all projections.
  - Output partials are written f16 (halves the out DMA).
"""

import math

import ml_dtypes
import numpy as np

BF16NP = ml_dtypes.bfloat16

import concourse.bass as bass
from concourse import bacc
import concourse.mybir as mybir
import concourse.tile as tile
from concourse.bass_utils import run_bass_kernel_spmd

F32 = mybir.dt.float32
F16 = mybir.dt.float16
AF = mybir.ActivationFunctionType
ALU = mybir.AluOpType
BF16 = mybir.dt.bfloat16

B, T, D, H = 2, 2048, 1024, 16
NCORES = 8
G = 4             # head groups (tensor parallel); cores = B * G
HPG = H // G      # 4 heads per core
DH = D // H       # 64 head dim
E = D // G        # 256 features per core
EB = E // 128     # 2 e-blocks of 128 (= head pairs)
KD = D // 128     # 8 contraction chunks for projections
TN = T // 512     # 4 512-wide stripes
TC = T // 128     # 16 128-wide k/t chunks


def build_nc():
    nc = bacc.Bacc(None)
    xqT = nc.declare_dram_parameter("xqT", [D, T], BF16, isOutput=False)
    xkT = nc.declare_dram_parameter("xkT", [D, T], BF16, isOutput=False)
    xvT = nc.declare_dram_parameter("xvT", [D, T], BF16, isOutput=False)
    wqT = nc.declare_dram_parameter("wqT", [D, E], BF16, isOutput=False)
    wkT = nc.declare_dram_parameter("wkT", [D, E], BF16, isOutput=False)
    wvT = nc.declare_dram_parameter("wvT", [D, E], BF16, isOutput=False)
    woT = nc.declare_dram_parameter("woT", [E, D], BF16, isOutput=False)
    outp = nc.declare_dram_parameter("outp", [T, D], F16, isOutput=True)

    with tile.TileContext(nc) as tc:
        with (
            tc.tile_pool(name="persist", bufs=1) as persist,
            tc.tile_pool(name="xs", bufs=12) as xs,
            tc.tile_pool(name="pt", bufs=12) as ptp,
            tc.tile_pool(name="rs", bufs=6) as rsp,
            tc.tile_pool(name="outs", bufs=4) as outsp,
            tc.tile_pool(name="psa", bufs=1, space="PSUM") as psa,
            tc.tile_pool(name="psb", bufs=1, space="PSUM") as psb,
            tc.tile_pool(name="po", bufs=1, space="PSUM") as pop,
            tc.tile_pool(name="pd", bufs=1, space="PSUM") as pdp,
            tc.tile_pool(name="pj", bufs=2, space="PSUM") as pjp,
        ):
            wq_sb = persist.tile([128, KD, E], BF16, tag="wq")
            wk_sb = persist.tile([128, KD, E], BF16, tag="wk")
            wv_sb = persist.tile([128, KD, E], BF16, tag="wv")
            wo_sb = persist.tile([128, EB, D], BF16, tag="wo")
            ones64 = persist.tile([128, 64], BF16, tag="ones")

            # fine-grained persist tiles so interleaved consumers only
            # depend on the exact producer stripe
            QTn = [[persist.tile([128, 512], BF16, tag=f"qt{eb}{n}", name=f"qt{eb}{n}")
                    for n in range(TN)] for eb in range(EB)]
            KTn = [[persist.tile([128, 512], BF16, tag=f"kt{eb}{n}", name=f"kt{eb}{n}")
                    for n in range(TN)] for eb in range(EB)]
            ONn = [[persist.tile([128, 512], BF16, tag=f"on{eb}{n}", name=f"on{eb}{n}")
                    for n in range(TN)] for eb in range(EB)]
            Vpn = [persist.tile([128, 4, HPG, DH], BF16, tag=f"vp{n}", name=f"vp{n}")
                   for n in range(TN)]

            nc.vector.memset(ones64[:], 1.0)
            # dummy exp: pulls the ACT table load off the first real
            # exp call's critical path
            dum = persist.tile([1, 2], BF16, tag="dum")
            nc.scalar.activation(dum[:, :], ones64[0:1, 0:2], AF.Exp)
            # HAM warm-up: keep the PE busy through the input-DMA window so
            # its clock gate is at 8/8 (2.4 GHz) when the first projection
            # matmuls arrive (the activity window needs ~3.4us of load)
            wup = pjp.tile([128, 64], F32, tag="pj")
            for _ in range(80):
                nc.tensor.matmul(wup[0:64, :], ones64[:, :], ones64[:, :],
                                 start=True, stop=True)

            def emit_x_dma(xdram, n):
                t = xs.tile([128, KD, 512], BF16, tag="x")
                nc.sync.dma_start(
                    t[:], xdram[:, :].rearrange("(c p) t -> p c t", p=128)[
                        :, :, 512 * n : 512 * n + 512]
                )
                return t

            def emit_qk_proj(eb, n, xt, wsb, dest):
                # medium priority (below S, above PV/den/out-proj backlog):
                # these produce the next attention block's inputs and must
                # not drain behind slack work
                acc = pjp.tile([128, 512], F32, tag="pj")
                with tc.high_priority(offset=25000):
                    for kd in range(KD):
                        nc.tensor.matmul(
                            acc[:],
                            wsb[:, kd, 128 * eb : 128 * eb + 128],
                            xt[:, kd, :],
                            start=(kd == 0),
                            stop=(kd == KD - 1),
                        )
                    nc.vector.tensor_copy(dest[eb][n][:, :], acc[:])

            def emit_v_proj_sub(n, sub, xt):
                acc = pjp.tile([128, E], F32, tag="pj")
                for kd in range(KD):
                    nc.tensor.matmul(
                        acc[:],
                        xt[:, kd, 128 * sub : 128 * sub + 128],
                        wv_sb[:, kd, :],
                        start=(kd == 0),
                        stop=(kd == KD - 1),
                    )
                nc.vector.tensor_copy(
                    Vpn[n][:, sub, :, :],
                    acc[:].rearrange("p (h d) -> p h d", h=HPG),
                )

            def emit_attn_jg(eb, jg, filler):
                """Attention for head pair eb, query stripe jg (512 q's),
                accumulating over k-chunks 0..4*jg+3 in pairs."""
                nkc = 4 * jg + 4
                pO = pop.tile([128, 512], F32, tag="po")
                pD = pdp.tile([128, 512], F32, tag="pd")

                # one k-chunk per step: pS is 2 banks so the pool's 2 bufs
                # let S(s+1) run during exp(s) — no exp<->S serial chain.
                # One exp call covers both heads, so all four col matmuls of
                # a step are co-ready: PV_h pairs with den of the OTHER head
                # (disjoint PE col groups) and the scheduler pops them
                # concurrently.  S is emitted one step ahead of the col
                # waves so the scalar engine's next input never queues
                # behind PV/den/filler work on the PE.
                def emit_S(kc):
                    # alternate between two single-buffer pools so each
                    # step's WAR dep is exactly exp(s-2), not batched with
                    # the other slot's chain
                    pool = psa if kc % 2 == 0 else psb
                    pS = pool.tile([128, 1024], F32, tag="ps", name="pS")
                    # highest scheduler priority: S feeds the scalar engine
                    # (exp), which must never starve behind projection /
                    # PV backlog on the PE; data deps still gate execution
                    with tc.high_priority(offset=50000):
                        for h in range(2):
                            r0 = 64 * h
                            nc.tensor.matmul(
                                pS[:, 512 * h : 512 * h + 512],
                                KTn[eb][kc // 4][r0 : r0 + 64,
                                                 128 * (kc % 4) : 128 * (kc % 4) + 128],
                                QTn[eb][jg][r0 : r0 + 64, :],
                                start=True,
                                stop=True,
                            )
                    return pS

                pS_next = emit_S(0)
                for kc in range(nkc):
                    pS = pS_next
                    ptb = ptp.tile([128, 1024], BF16, tag="pt")
                    nc.scalar.activation(ptb[:, :], pS[:, :], AF.Exp)
                    if kc >= 4 * jg:  # diagonal chunk: causal mask in place
                        # one call over both heads (outer level restarts the
                        # q ramp per 512-half) so they stay co-ready
                        nc.gpsimd.affine_select(
                            out=ptb[:, :].rearrange("p (h q) -> p h q", h=2),
                            in_=ptb[:, :].rearrange("p (h q) -> p h q", h=2),
                            pattern=[[0, 2], [1, 512]],
                            compare_op=ALU.is_ge,
                            fill=0.0,
                            base=-(128 * (kc % 4)),
                            channel_multiplier=-1,
                        )
                    if kc + 1 < nkc:
                        pS_next = emit_S(kc + 1)

                    def pv_mm(h):
                        nc.tensor.matmul(
                            pO[64 * h : 64 * h + 64, :],
                            Vpn[kc // 4][:, kc % 4, 2 * eb + h, :],
                            ptb[:, 512 * h : 512 * h + 512],
                            start=(kc == 0),
                            stop=(kc == nkc - 1),
                            skip_group_check=True,
                        )
                    def den_mm(h):
                        nc.tensor.matmul(
                            pD[64 * h : 64 * h + 64, :],
                            ones64[:, :],
                            ptb[:, 512 * h : 512 * h + 512],
                            start=(kc == 0),
                            stop=(kc == nkc - 1),
                            skip_group_check=True,
                        )
                    pv_mm(0); den_mm(1)
                    pv_mm(1); den_mm(0)
                    if filler:
                        filler.pop(0)()
                # normalization: recip of broadcast denominator, then scale.
                # fast 51-ULP recip (1 DVE inst) is plenty for the 2e-2
                # budget.  The last block's norm is split in halves so the
                # tail out-projections start as soon as their half lands.
                r = rsp.tile([128, 512], F32, tag="rs")
                halves = 2 if (eb, jg) == (1, 3) else 1
                w = 512 // halves
                for hv in range(halves):
                    sl = slice(w * hv, w * hv + w)
                    nc.vector.reciprocal_approx_fast(out=r[:, sl], in_=pD[:, sl])
                    nc.vector.tensor_tensor(
                        out=ONn[eb][jg][:, sl], in0=pO[:, sl], in1=r[:, sl],
                        op=ALU.mult,
                    )

            def emit_outproj(tn, tail=False):
                obt = outsp.tile([128, 1024], F16, tag="ob")
                for dn in range(2):
                    acc = pjp.tile([128, 512], F32, tag="pj")
                    for eb in range(EB):
                        nc.tensor.matmul(
                            acc[:],
                            ONn[eb][tn // 4][:, 128 * (tn % 4) : 128 * (tn % 4) + 128],
                            wo_sb[:, eb, 512 * dn : 512 * dn + 512],
                            start=(eb == 0),
                            stop=(eb == EB - 1),
                        )
                    if tail and dn == 1:
                        # after the last exp, the scalar engine is free:
                        # parallelize the final casts across engines
                        nc.scalar.copy(obt[:, 512:1024], acc[:])
                    else:
                        nc.vector.tensor_copy(obt[:, 512 * dn : 512 * dn + 512], acc[:])
                nc.sync.dma_start(
                    outp[128 * tn : 128 * tn + 128, :], obt[:]
                )

            # ---- emission schedule ----
            # DMA order front-loads exactly what the first projections need
            nc.sync.dma_start(wq_sb[:], wqT[:, :].rearrange("(c p) e -> p c e", p=128))
            xq0 = emit_x_dma(xqT, 0)
            nc.sync.dma_start(wk_sb[:], wkT[:, :].rearrange("(c p) e -> p c e", p=128))
            xk0 = emit_x_dma(xkT, 0)
            nc.sync.dma_start(wv_sb[:], wvT[:, :].rearrange("(c p) e -> p c e", p=128))
            xv0 = emit_x_dma(xvT, 0)
            nc.sync.dma_start(wo_sb[:], woT[:, :].rearrange("(c p) d -> p c d", p=128))
            emit_qk_proj(0, 0, xq0, wq_sb, QTn)
            emit_qk_proj(0, 0, xk0, wk_sb, KTn)
            for sub in range(4):
                emit_v_proj_sub(0, sub, xv0)
            # prefetch the remaining stripes now: the DMA queue streams them
            # during stripe-0 compute so later projections never wait
            xst = {}
            for n in range(1, TN):
                xst[n] = (emit_x_dma(xqT, n), emit_x_dma(xkT, n),
                          emit_x_dma(xvT, n))

            # interleave head pairs per q-stripe; ALL remaining projection
            # and out-projection work rides as fillers inside attention
            # steps so no emission block ever starves the scalar engine
            op = lambda t: (lambda: emit_outproj(t))
            def q0(m):
                return lambda: emit_qk_proj(0, m, xst[m][0], wq_sb, QTn)
            def k0(m):
                return lambda: emit_qk_proj(0, m, xst[m][1], wk_sb, KTn)
            def q1(m):
                x = xq0 if m == 0 else xst[m][0]
                return lambda: emit_qk_proj(1, m, x, wq_sb, QTn)
            def k1(m):
                x = xk0 if m == 0 else xst[m][1]
                return lambda: emit_qk_proj(1, m, x, wk_sb, KTn)
            def vp(m, s0):
                return lambda: (emit_v_proj_sub(m, s0, xst[m][2]),
                                emit_v_proj_sub(m, s0 + 1, xst[m][2]))

            emit_attn_jg(0, 0, [q1(0), k1(0)])
            emit_attn_jg(1, 0, [q0(1), k0(1), vp(1, 0), vp(1, 2)])
            emit_attn_jg(0, 1, [q1(1), k1(1)])
            emit_attn_jg(1, 1, [q0(2), k0(2), vp(2, 0), vp(2, 2),
                                op(0), op(1), op(2), op(3)])
            emit_attn_jg(0, 2, [q1(2), k1(2)])
            emit_attn_jg(1, 2, [q0(3), k0(3), vp(3, 0), vp(3, 2),
                                op(4), op(5), op(6), op(7)])
            emit_attn_jg(0, 3, [q1(3), k1(3)])
            emit_attn_jg(1, 3, [op(t) for t in range(8, 12)])
            for t in range(12, 16):
                emit_outproj(t, tail=True)
    nc.compile()
    return nc


_CACHE = {}
LAST_RESULTS = None


def get_nc():
    if "nc" not in _CACHE:
        _CACHE["nc"] = build_nc()
    return _CACHE["nc"]


def make_in_maps(q, k, v, wq, wk, wv, wo):
    q, k, v, wq, wk, wv, wo = (
        np.asarray(a, dtype=np.float32) for a in (q, k, v, wq, wk, wv, wo)
    )
    scale = 1.0 / math.sqrt(DH)
    xT = [
        (
            np.ascontiguousarray(q[b].T).astype(BF16NP),
            np.ascontiguousarray(k[b].T).astype(BF16NP),
            np.ascontiguousarray(v[b].T).astype(BF16NP),
        )
        for b in range(B)
    ]
    in_maps = []
    for c in range(NCORES):
        b, g = divmod(c, G)
        gs = slice(E * g, E * (g + 1))
        in_maps.append(
            {
                "xqT": xT[b][0],
                "xkT": xT[b][1],
                "xvT": xT[b][2],
                "wqT": np.ascontiguousarray((wq[gs] * scale).T).astype(BF16NP),
                "wkT": np.ascontiguousarray(wk[gs].T).astype(BF16NP),
                "wvT": np.ascontiguousarray(wv[gs].T).astype(BF16NP),
                "woT": np.ascontiguousarray(wo[:, gs].T).astype(BF16NP),
            }
        )
    return in_maps


def kernel(q, k, v, wq, wk, wv, wo):
    global LAST_RESULTS
    nc = get_nc()
    in_maps = make_in_maps(q, k, v, wq, wk, wv, wo)
    res = run_bass_kernel_spmd(nc, in_maps, core_ids=list(range(NCORES)))
    LAST_RESULTS = res
    out = np.zeros((B, T, D), dtype=np.float32)
    for c in range(NCORES):
        out[c // G] += np.asarray(res.results[c]["outp"], dtype=np.float32)
    return out
